# revision 10
# baseline (speedup 1.0000x reference)
"""Trainium2 Bass kernel for CustomConv2d:
  x [16, 32, 512, 512] f32, weight [32, 32, 3, 3] f32, bias [32] f32
  -> out [16, 32, 510, 510] f32   (stride 1, VALID padding, + bias)

Data-parallel over batch: 2 images per core across 8 NeuronCores.

v6 design — bf16 HBM I/O, host-side swizzle, M=64 tap-pair matmuls:
 - host pre-swizzles x into the SBUF strip layout, bf16 (4KB descriptors,
   two 0.5MB dma_starts per strip):
   xs[img*16+s, 32g+ci, 512k+w] = x[img, ci, 32s+4k+g, w]
 - mod-4 row rotation: partition group g holds rows r = g (mod 4).
 - tap-row pairs: one K=64 M=64 matmul streams an adjacent row pair
   (g0,g1 or g2,g3) once and feeds TWO output rows.  Per out-row quad
   (4 rows, bank-aligned): psA (T0 rows) and psB (T1 rows) each take
   6 MMs (3 kw x 2 col-halves).  12 MMs per quad instead of 24 —
   halves both the matmul count and the LDWEIGHTS occupancy that bound
   v5 (uniform 64x64 tiling, no mode switches).
     psA_q: tile(0,0)=MM_a(pair@q), tile(0,64)=MM_b(pair@q+1)
     psB_q: tile(64,64)=MM_a(pair@q), tile(64,0)=MM_b(pair@q+1)
   pattern A: cols(out r):   g_lo=kh0, g_hi=kh1; cols(out r+1): g_lo=0,  g_hi=kh0
   pattern B: cols(out r-2): g_lo=kh2, g_hi=0;   cols(out r-1): g_lo=kh1, g_hi=kh2
 - drain: ACT Identity(psB + bias) -> t (f32); DVE t + psA -> ostrip bf16.
 - output bf16: ostrip[32p+co, 510*q + w] = out row 32s+4q+p; one 1MB
   dma_start per strip (8160B descriptors); host de-swizzles + upcasts.
"""
import numpy as np
import ml_dtypes

import concourse.bass as bass
import concourse.tile as tile
from concourse import bacc, mybir
from concourse.bass_utils import run_bass_kernel_spmd
from contextlib import ExitStack

F32 = mybir.dt.float32
BF16 = mybir.dt.bfloat16
BF = ml_dtypes.bfloat16

N_FULL, C, H, W = 16, 32, 512, 512
HO = WO = 510
N_CORES = 8
N_PER = N_FULL // N_CORES
N_STRIPS = H // 32
NS = N_PER * N_STRIPS


def _build():
    nc = bacc.Bacc("TRN2", target_bir_lowering=False, debug=False, num_devices=1)
    x_d = nc.dram_tensor("xs", [NS, 128, 4096], BF16, kind="ExternalInput").ap()
    w_d = nc.dram_tensor("wb", [128, 384], BF16, kind="ExternalInput").ap()
    b_d = nc.dram_tensor("bt", [128, 1], F32, kind="ExternalInput").ap()
    o_d = nc.dram_tensor("out", [NS, 128, 4080], BF16, kind="ExternalOutput").ap()

    with tile.TileContext(nc) as tc, ExitStack() as ctx:
        const_pool = ctx.enter_context(tc.tile_pool(name="const", bufs=1))
        xb_pool = ctx.enter_context(tc.tile_pool(name="xb", bufs=6))
        psum_pool = ctx.enter_context(tc.tile_pool(name="ps", bufs=4, space="PSUM"))
        t_pool = ctx.enter_context(tc.tile_pool(name="t", bufs=6))
        out_pool = ctx.enter_context(tc.tile_pool(name="ostrip", bufs=4))

        wb = const_pool.tile([128, 384], BF16)
        nc.sync.dma_start(wb[:], w_d[:])
        bt = const_pool.tile([128, 1], F32)
        nc.scalar.dma_start(bt[:], b_d[:])

        engs = [nc.sync, nc.scalar, nc.gpsimd]
        rr = [0]

        def in_dma(dst, src):
            engs[rr[0] % 3].dma_start(dst, src)
            rr[0] += 1

        def out_dma(dst, src):
            engs[rr[0] % 3].dma_start(dst, src)
            rr[0] += 1

        def skew():
            # 2 reads + 1 write advance rr by 3 per strip; extra bump so
            # each queue rotates through both roles (balances bytes/queue)
            rr[0] += 1

        uid = [0]

        # weight column layout: 32*(ab*6 + kw*2 + colhalf) for ab in {A=0,B=1}
        def wcol(ab, kw, ch):
            return 32 * (ab * 6 + kw * 2 + ch)

        def emit_quad(q, xcur, xnext, ostrip, nrow=4):
            """One out-row quad (rows y0..y0+3, y0 = 32s + 4q).
            MM_a from pair@slot q, MM_b from pair@slot q+1 (may be xnext).
            nrow=2 for the final quad of an image (skips MM_b side and
            drains only partitions 0..63)."""
            uid[0] += 1
            psA = psum_pool.tile([128, 512], F32, tag="psA",
                                 name=f"psA_{uid[0]}")
            psB = psum_pool.tile([128, 512], F32, tag="psB",
                                 name=f"psB_{uid[0]}")
            xa_b = xnext if q == 7 else xcur
            sl_b = 0 if q == 7 else q + 1
            for kw in range(3):
                # (bank, row-half T, a/b, col tile, x tile, slot)
                # psA: MM_a = T0 pair of this quad, MM_b = T0 pair of the
                # NEXT quad (rows 4Q+4,4Q+5).  psB: both MMs use this
                # quad's T1 pair (rows 4Q+2,4Q+3).
                mms = [(psA, 0, 0, 0, xcur, q)]
                if nrow == 4:
                    mms += [(psB, 1, 0, 64, xcur, q),
                            (psA, 0, 1, 64, xa_b, sl_b)]
                mms.append((psB, 1, 1, 0, xcur, q))
                for ps, T, ab, ct, xa, sl in mms:
                    base = 64 * T
                    off = sl * 512 + kw
                    nc.tensor.matmul(
                        ps[ct:ct + 64, 0:WO],
                        wb[base:base + 64, wcol(ab, kw, 0):wcol(ab, kw, 0) + 64],
                        xa[base:base + 64, off:off + WO],
                        start=(kw == 0), stop=(kw == 2),
                        skip_group_check=True,
                        tile_position=(base, ct),
                    )
            npart = 32 * nrow
            uid[0] += 1
            t = t_pool.tile([128, WO], BF16, tag="t", name=f"t_{uid[0]}")
            nc.scalar.activation(t[0:npart, :], psB[0:npart, 0:WO],
                                 mybir.ActivationFunctionType.Identity,
                                 bias=bt[0:npart, :])
            nc.vector.tensor_add(
                ostrip[0:npart, 510 * q:510 * q + WO],
                t[0:npart, :], psA[0:npart, 0:WO])

        for n in range(N_PER):
            xb = {}
            ost = {}
            for s in range(N_STRIPS):
                uid[0] += 1
                xb[s] = xb_pool.tile([128, 4096], BF16, tag="xb",
                                     name=f"xb_{uid[0]}")
                in_dma(xb[s][:, 0:2048], x_d[n * N_STRIPS + s, :, 0:2048])
                in_dma(xb[s][:, 2048:4096], x_d[n * N_STRIPS + s, :, 2048:4096])
                skew()
                uid[0] += 1
                ost[s] = out_pool.tile([128, 4080], BF16, tag="ostrip",
                                       name=f"os_{uid[0]}")
                if s >= 1:
                    for q in range(8):
                        emit_quad(q, xb[s - 1], xb[s] if q == 7 else None,
                                  ost[s - 1])
                    out_dma(o_d[n * N_STRIPS + s - 1], ost[s - 1][:])
            s = N_STRIPS - 1
            for q in range(8):
                emit_quad(q, xb[s], None, ost[s], nrow=4 if q < 7 else 2)
            idx = n * N_STRIPS + s
            out_dma(o_d[idx, 0:64], ost[s][0:64, :])
            out_dma(o_d[idx, 64:128, 0:3570], ost[s][64:128, 0:3570])

    nc.compile()
    return nc


def _prep_x(x):
    """[16, 32, 512, 512] f32 -> per-core list of [32, 128, 4096] bf16."""
    xb = x.astype(BF)
    cores = []
    for c in range(N_CORES):
        imgs = []
        for n in range(N_PER):
            im = xb[c * N_PER + n]                      # [32, 512, 512]
            im = im.reshape(C, N_STRIPS, 8, 4, W)       # ci, s, k, g, w
            im = im.transpose(1, 3, 0, 2, 4)            # s, g, ci, k, w
            imgs.append(np.ascontiguousarray(im.reshape(N_STRIPS, 128, 4096)))
        cores.append(np.concatenate(imgs, axis=0))
    return cores


def _prep_w(weight):
    """[32, 32, 3, 3] f32 -> [128, 384] bf16.
    Column layout: 32*(ab*6 + kw*2 + colhalf); both K-halves (partitions
    0-63 and 64-127) carry the same content.
      A: [[kh0, 0], [kh1, kh0]]   (K-half x col-half)
      B: [[kh2, kh1], [0, kh2]]
    """
    wb = np.zeros((128, 384), dtype=np.float32)
    wt = {kh: weight[:, :, kh, :] for kh in range(3)}
    for kw in range(3):
        for T in (0, 64):
            for ab, pat in ((0, ((0, None), (1, 0))), (1, ((2, 1), (None, 2)))):
                c0 = 32 * (ab * 6 + kw * 2)
                for gl in range(2):          # K sub-half (g_lo, g_hi)
                    for ch in range(2):      # col half (out row 0/1 of pair)
                        kh = pat[gl][ch]
                        if kh is not None:
                            wb[T + 32 * gl:T + 32 * gl + 32,
                               c0 + 32 * ch:c0 + 32 * ch + 32] = \
                                weight[:, :, kh, kw].T
    return wb.astype(BF)


def _unprep_out(o_arrs):
    """per-core [32, 128, 4080] bf16 -> [16, 32, 510, 510] f32."""
    full = np.empty((N_FULL, C, HO, WO), dtype=np.float32)
    for c, arr in enumerate(o_arrs):
        a = np.asarray(arr).reshape(N_PER, N_STRIPS, 4, 32, 8, WO)
        # dims: n, s, p, co, q, w  ->  n, co, s, q, p, w
        a = a.transpose(0, 3, 1, 4, 2, 5).reshape(N_PER, C, 512, WO)
        full[c * N_PER:(c + 1) * N_PER] = a[:, :, :HO, :].astype(np.float32)
    return full


_NC = None


def prepare_in_maps(x, weight, bias):
    x = np.ascontiguousarray(np.asarray(x, dtype=np.float32))
    weight = np.ascontiguousarray(np.asarray(weight, dtype=np.float32))
    bias = np.ascontiguousarray(np.asarray(bias, dtype=np.float32))
    xs = _prep_x(x)
    wb = _prep_w(weight)
    bt = np.repeat(bias.reshape(1, 32), 4, axis=0).reshape(128, 1)
    bt = np.ascontiguousarray(bt, dtype=np.float32)
    return [{"xs": xs[i], "wb": wb, "bt": bt} for i in range(N_CORES)]


def kernel(x, weight, bias):
    global _NC
    if _NC is None:
        _NC = _build()
    in_maps = prepare_in_maps(x, weight, bias)
    res = run_bass_kernel_spmd(_NC, in_maps, core_ids=list(range(N_CORES)))
    return _unprep_out([r["out"] for r in res.results])


# revision 11
# speedup vs baseline: 1.0210x; 1.0210x over previous
"""Trainium2 Bass kernel for CustomConv2d:
  x [16, 32, 512, 512] f32, weight [32, 32, 3, 3] f32, bias [32] f32
  -> out [16, 32, 510, 510] f32   (stride 1, VALID padding, + bias)

Data-parallel over batch: 2 images per core across 8 NeuronCores.

v6 design — bf16 HBM I/O, host-side swizzle, M=64 tap-pair matmuls:
 - host pre-swizzles x into the SBUF strip layout, bf16 (4KB descriptors,
   two 0.5MB dma_starts per strip):
   xs[img*16+s, 32g+ci, 512k+w] = x[img, ci, 32s+4k+g, w]
 - mod-4 row rotation: partition group g holds rows r = g (mod 4).
 - tap-row pairs: one K=64 M=64 matmul streams an adjacent row pair
   (g0,g1 or g2,g3) once and feeds TWO output rows.  Per out-row quad
   (4 rows, bank-aligned): psA (T0 rows) and psB (T1 rows) each take
   6 MMs (3 kw x 2 col-halves).  12 MMs per quad instead of 24 —
   halves both the matmul count and the LDWEIGHTS occupancy that bound
   v5 (uniform 64x64 tiling, no mode switches).
     psA_q: tile(0,0)=MM_a(pair@q), tile(0,64)=MM_b(pair@q+1)
     psB_q: tile(64,64)=MM_a(pair@q), tile(64,0)=MM_b(pair@q+1)
   pattern A: cols(out r):   g_lo=kh0, g_hi=kh1; cols(out r+1): g_lo=0,  g_hi=kh0
   pattern B: cols(out r-2): g_lo=kh2, g_hi=0;   cols(out r-1): g_lo=kh1, g_hi=kh2
 - drain: ACT Identity(psB + bias) -> t (f32); DVE t + psA -> ostrip bf16.
 - output bf16: ostrip[32p+co, 510*q + w] = out row 32s+4q+p; one 1MB
   dma_start per strip (8160B descriptors); host de-swizzles + upcasts.
"""
import numpy as np
import ml_dtypes

import concourse.bass as bass
import concourse.tile as tile
from concourse import bacc, mybir
from concourse.bass_utils import run_bass_kernel_spmd
from contextlib import ExitStack

F32 = mybir.dt.float32
BF16 = mybir.dt.bfloat16
BF = ml_dtypes.bfloat16

N_FULL, C, H, W = 16, 32, 512, 512
HO = WO = 510
N_CORES = 8
N_PER = N_FULL // N_CORES
N_STRIPS = H // 32
NS = N_PER * N_STRIPS


def _build():
    nc = bacc.Bacc("TRN2", target_bir_lowering=False, debug=False, num_devices=1)
    x_d = nc.dram_tensor("xs", [NS, 128, 4096], BF16, kind="ExternalInput").ap()
    w_d = nc.dram_tensor("wb", [128, 384], BF16, kind="ExternalInput").ap()
    b_d = nc.dram_tensor("bt", [128, 1], F32, kind="ExternalInput").ap()
    o_d = nc.dram_tensor("out", [NS, 128, 4080], BF16, kind="ExternalOutput").ap()

    with tile.TileContext(nc) as tc, ExitStack() as ctx:
        const_pool = ctx.enter_context(tc.tile_pool(name="const", bufs=1))
        xb_pool = ctx.enter_context(tc.tile_pool(name="xb", bufs=6))
        psum_pool = ctx.enter_context(tc.tile_pool(name="ps", bufs=4, space="PSUM"))
        t_pool = ctx.enter_context(tc.tile_pool(name="t", bufs=6))
        out_pool = ctx.enter_context(tc.tile_pool(name="ostrip", bufs=4))

        wb = const_pool.tile([128, 384], BF16)
        nc.sync.dma_start(wb[:], w_d[:])
        bt = const_pool.tile([128, 1], F32)
        nc.scalar.dma_start(bt[:], b_d[:])

        engs = [nc.sync, nc.scalar, nc.gpsimd]
        rr = [0]

        def in_dma(dst, src):
            engs[rr[0] % 3].dma_start(dst, src)
            rr[0] += 1

        def out_dma(dst, src):
            engs[(rr[0] + 1) % 3].dma_start(dst, src)
            rr[0] += 1

        uid = [0]

        # weight column layout: 32*(ab*6 + kw*2 + colhalf) for ab in {A=0,B=1}
        def wcol(ab, kw, ch):
            return 32 * (ab * 6 + kw * 2 + ch)

        def emit_quad(q, xcur, xnext, ostrip, nrow=4):
            """One out-row quad (rows y0..y0+3, y0 = 32s + 4q).
            MM_a from pair@slot q, MM_b from pair@slot q+1 (may be xnext).
            nrow=2 for the final quad of an image (skips MM_b side and
            drains only partitions 0..63)."""
            uid[0] += 1
            psA = psum_pool.tile([128, 512], F32, tag="psA",
                                 name=f"psA_{uid[0]}")
            psB = psum_pool.tile([128, 512], F32, tag="psB",
                                 name=f"psB_{uid[0]}")
            xa_b = xnext if q == 7 else xcur
            sl_b = 0 if q == 7 else q + 1
            for kw in range(3):
                # (bank, row-half T, a/b, col tile, x tile, slot)
                # psA: MM_a = T0 pair of this quad, MM_b = T0 pair of the
                # NEXT quad (rows 4Q+4,4Q+5).  psB: both MMs use this
                # quad's T1 pair (rows 4Q+2,4Q+3).
                mms = [(psA, 0, 0, 0, xcur, q)]
                if nrow == 4:
                    mms += [(psB, 1, 0, 64, xcur, q),
                            (psA, 0, 1, 64, xa_b, sl_b)]
                mms.append((psB, 1, 1, 0, xcur, q))
                for ps, T, ab, ct, xa, sl in mms:
                    base = 64 * T
                    off = sl * 512 + kw
                    nc.tensor.matmul(
                        ps[ct:ct + 64, 0:WO],
                        wb[base:base + 64, wcol(ab, kw, 0):wcol(ab, kw, 0) + 64],
                        xa[base:base + 64, off:off + WO],
                        start=(kw == 0), stop=(kw == 2),
                        skip_group_check=True,
                        tile_position=(base, ct),
                    )
            npart = 32 * nrow
            uid[0] += 1
            t = t_pool.tile([128, WO], F32, tag="t", name=f"t_{uid[0]}")
            nc.scalar.activation(t[0:npart, :], psB[0:npart, 0:WO],
                                 mybir.ActivationFunctionType.Identity,
                                 bias=bt[0:npart, :])
            nc.vector.tensor_add(
                ostrip[0:npart, 510 * q:510 * q + WO],
                t[0:npart, :], psA[0:npart, 0:WO])

        for n in range(N_PER):
            xb = {}
            ost = {}
            for s in range(N_STRIPS):
                uid[0] += 1
                xb[s] = xb_pool.tile([128, 4096], BF16, tag="xb",
                                     name=f"xb_{uid[0]}")
                in_dma(xb[s][:, 0:2048], x_d[n * N_STRIPS + s, :, 0:2048])
                in_dma(xb[s][:, 2048:4096], x_d[n * N_STRIPS + s, :, 2048:4096])
                uid[0] += 1
                ost[s] = out_pool.tile([128, 4080], BF16, tag="ostrip",
                                       name=f"os_{uid[0]}")
                if s >= 1:
                    for q in range(8):
                        emit_quad(q, xb[s - 1], xb[s] if q == 7 else None,
                                  ost[s - 1])
                    out_dma(o_d[n * N_STRIPS + s - 1], ost[s - 1][:])
            s = N_STRIPS - 1
            for q in range(8):
                emit_quad(q, xb[s], None, ost[s], nrow=4 if q < 7 else 2)
            idx = n * N_STRIPS + s
            out_dma(o_d[idx, 0:64], ost[s][0:64, :])
            out_dma(o_d[idx, 64:128, 0:3570], ost[s][64:128, 0:3570])

    nc.compile()
    return nc


def _prep_x(x):
    """[16, 32, 512, 512] f32 -> per-core list of [32, 128, 4096] bf16."""
    xb = x.astype(BF)
    cores = []
    for c in range(N_CORES):
        imgs = []
        for n in range(N_PER):
            im = xb[c * N_PER + n]                      # [32, 512, 512]
            im = im.reshape(C, N_STRIPS, 8, 4, W)       # ci, s, k, g, w
            im = im.transpose(1, 3, 0, 2, 4)            # s, g, ci, k, w
            imgs.append(np.ascontiguousarray(im.reshape(N_STRIPS, 128, 4096)))
        cores.append(np.concatenate(imgs, axis=0))
    return cores


def _prep_w(weight):
    """[32, 32, 3, 3] f32 -> [128, 384] bf16.
    Column layout: 32*(ab*6 + kw*2 + colhalf); both K-halves (partitions
    0-63 and 64-127) carry the same content.
      A: [[kh0, 0], [kh1, kh0]]   (K-half x col-half)
      B: [[kh2, kh1], [0, kh2]]
    """
    wb = np.zeros((128, 384), dtype=np.float32)
    wt = {kh: weight[:, :, kh, :] for kh in range(3)}
    for kw in range(3):
        for T in (0, 64):
            for ab, pat in ((0, ((0, None), (1, 0))), (1, ((2, 1), (None, 2)))):
                c0 = 32 * (ab * 6 + kw * 2)
                for gl in range(2):          # K sub-half (g_lo, g_hi)
                    for ch in range(2):      # col half (out row 0/1 of pair)
                        kh = pat[gl][ch]
                        if kh is not None:
                            wb[T + 32 * gl:T + 32 * gl + 32,
                               c0 + 32 * ch:c0 + 32 * ch + 32] = \
                                weight[:, :, kh, kw].T
    return wb.astype(BF)


def _unprep_out(o_arrs):
    """per-core [32, 128, 4080] bf16 -> [16, 32, 510, 510] f32."""
    full = np.empty((N_FULL, C, HO, WO), dtype=np.float32)
    for c, arr in enumerate(o_arrs):
        a = np.asarray(arr).reshape(N_PER, N_STRIPS, 4, 32, 8, WO)
        # dims: n, s, p, co, q, w  ->  n, co, s, q, p, w
        a = a.transpose(0, 3, 1, 4, 2, 5).reshape(N_PER, C, 512, WO)
        full[c * N_PER:(c + 1) * N_PER] = a[:, :, :HO, :].astype(np.float32)
    return full


_NC = None


def prepare_in_maps(x, weight, bias):
    x = np.ascontiguousarray(np.asarray(x, dtype=np.float32))
    weight = np.ascontiguousarray(np.asarray(weight, dtype=np.float32))
    bias = np.ascontiguousarray(np.asarray(bias, dtype=np.float32))
    xs = _prep_x(x)
    wb = _prep_w(weight)
    bt = np.repeat(bias.reshape(1, 32), 4, axis=0).reshape(128, 1)
    bt = np.ascontiguousarray(bt, dtype=np.float32)
    return [{"xs": xs[i], "wb": wb, "bt": bt} for i in range(N_CORES)]


def kernel(x, weight, bias):
    global _NC
    if _NC is None:
        _NC = _build()
    in_maps = prepare_in_maps(x, weight, bias)
    res = run_bass_kernel_spmd(_NC, in_maps, core_ids=list(range(N_CORES)))
    return _unprep_out([r["out"] for r in res.results])


# revision 12
# speedup vs baseline: 1.0478x; 1.0263x over previous
"""Trainium2 Bass kernel for CustomConv2d:
  x [16, 32, 512, 512] f32, weight [32, 32, 3, 3] f32, bias [32] f32
  -> out [16, 32, 510, 510] f32   (stride 1, VALID padding, + bias)

Data-parallel over batch: 2 images per core across 8 NeuronCores.

v6 design — bf16 HBM I/O, host-side swizzle, M=64 tap-pair matmuls:
 - host pre-swizzles x into the SBUF strip layout, bf16 (4KB descriptors,
   two 0.5MB dma_starts per strip):
   xs[img*16+s, 32g+ci, 512k+w] = x[img, ci, 32s+4k+g, w]
 - mod-4 row rotation: partition group g holds rows r = g (mod 4).
 - tap-row pairs: one K=64 M=64 matmul streams an adjacent row pair
   (g0,g1 or g2,g3) once and feeds TWO output rows.  Per out-row quad
   (4 rows, bank-aligned): psA (T0 rows) and psB (T1 rows) each take
   6 MMs (3 kw x 2 col-halves).  12 MMs per quad instead of 24 —
   halves both the matmul count and the LDWEIGHTS occupancy that bound
   v5 (uniform 64x64 tiling, no mode switches).
     psA_q: tile(0,0)=MM_a(pair@q), tile(0,64)=MM_b(pair@q+1)
     psB_q: tile(64,64)=MM_a(pair@q), tile(64,0)=MM_b(pair@q+1)
   pattern A: cols(out r):   g_lo=kh0, g_hi=kh1; cols(out r+1): g_lo=0,  g_hi=kh0
   pattern B: cols(out r-2): g_lo=kh2, g_hi=0;   cols(out r-1): g_lo=kh1, g_hi=kh2
 - drain: ACT Identity(psB + bias) -> t (f32); DVE t + psA -> ostrip bf16.
 - output bf16: ostrip[32p+co, 510*q + w] = out row 32s+4q+p; one 1MB
   dma_start per strip (8160B descriptors); host de-swizzles + upcasts.
"""
import numpy as np
import ml_dtypes

import concourse.bass as bass
import concourse.tile as tile
from concourse import bacc, mybir
from concourse.bass_utils import run_bass_kernel_spmd
from contextlib import ExitStack

F32 = mybir.dt.float32
BF16 = mybir.dt.bfloat16
BF = ml_dtypes.bfloat16

N_FULL, C, H, W = 16, 32, 512, 512
HO = WO = 510
N_CORES = 8
N_PER = N_FULL // N_CORES
N_STRIPS = H // 32
NS = N_PER * N_STRIPS


def _build():
    nc = bacc.Bacc("TRN2", target_bir_lowering=False, debug=False, num_devices=1)
    x_d = nc.dram_tensor("xs", [NS, 128, 4096], BF16, kind="ExternalInput").ap()
    w_d = nc.dram_tensor("wb", [128, 384], BF16, kind="ExternalInput").ap()
    b_d = nc.dram_tensor("bt", [128, 1], F32, kind="ExternalInput").ap()
    o_d = nc.dram_tensor("out", [NS, 128, 4080], BF16, kind="ExternalOutput").ap()

    with tile.TileContext(nc) as tc, ExitStack() as ctx:
        const_pool = ctx.enter_context(tc.tile_pool(name="const", bufs=1))
        xb_pool = ctx.enter_context(tc.tile_pool(name="xb", bufs=6))
        psum_pool = ctx.enter_context(tc.tile_pool(name="ps", bufs=4, space="PSUM"))
        t_pool = ctx.enter_context(tc.tile_pool(name="t", bufs=6))
        out_pool = ctx.enter_context(tc.tile_pool(name="ostrip", bufs=4))

        wb = const_pool.tile([128, 384], BF16)
        nc.gpsimd.dma_start(wb[:], w_d[:])
        bt = const_pool.tile([128, 1], F32)
        nc.gpsimd.dma_start(bt[:], b_d[:])

        # Reads own the two HWDGE queues (sync/scalar) so they are never
        # head-of-line blocked; writes (which wait on drains) ride the
        # async SWDGE (gpsimd) queue.
        rr = [0]

        def in_dma(dst, src):
            (nc.sync if rr[0] % 2 == 0 else nc.scalar).dma_start(dst, src)
            rr[0] += 1

        def out_dma(dst, src):
            nc.gpsimd.dma_start(dst, src)

        uid = [0]

        # weight column layout: 32*(ab*6 + kw*2 + colhalf) for ab in {A=0,B=1}
        def wcol(ab, kw, ch):
            return 32 * (ab * 6 + kw * 2 + ch)

        def emit_quad(q, xcur, xnext, ostrip, nrow=4):
            """One out-row quad (rows y0..y0+3, y0 = 32s + 4q).
            MM_a from pair@slot q, MM_b from pair@slot q+1 (may be xnext).
            nrow=2 for the final quad of an image (skips MM_b side and
            drains only partitions 0..63)."""
            uid[0] += 1
            psA = psum_pool.tile([128, 512], F32, tag="psA",
                                 name=f"psA_{uid[0]}")
            psB = psum_pool.tile([128, 512], F32, tag="psB",
                                 name=f"psB_{uid[0]}")
            xa_b = xnext if q == 7 else xcur
            sl_b = 0 if q == 7 else q + 1
            for kw in range(3):
                # (bank, row-half T, a/b, col tile, x tile, slot)
                # psA: MM_a = T0 pair of this quad, MM_b = T0 pair of the
                # NEXT quad (rows 4Q+4,4Q+5).  psB: both MMs use this
                # quad's T1 pair (rows 4Q+2,4Q+3).
                mms = [(psA, 0, 0, 0, xcur, q)]
                if nrow == 4:
                    mms += [(psB, 1, 0, 64, xcur, q),
                            (psA, 0, 1, 64, xa_b, sl_b)]
                mms.append((psB, 1, 1, 0, xcur, q))
                for ps, T, ab, ct, xa, sl in mms:
                    base = 64 * T
                    off = sl * 512 + kw
                    nc.tensor.matmul(
                        ps[ct:ct + 64, 0:WO],
                        wb[base:base + 64, wcol(ab, kw, 0):wcol(ab, kw, 0) + 64],
                        xa[base:base + 64, off:off + WO],
                        start=(kw == 0), stop=(kw == 2),
                        skip_group_check=True,
                        tile_position=(base, ct),
                    )
            npart = 32 * nrow
            uid[0] += 1
            t = t_pool.tile([128, WO], F32, tag="t", name=f"t_{uid[0]}")
            nc.scalar.activation(t[0:npart, :], psB[0:npart, 0:WO],
                                 mybir.ActivationFunctionType.Identity,
                                 bias=bt[0:npart, :])
            nc.vector.tensor_add(
                ostrip[0:npart, 510 * q:510 * q + WO],
                t[0:npart, :], psA[0:npart, 0:WO])

        for n in range(N_PER):
            xb = {}
            ost = {}
            for s in range(N_STRIPS):
                uid[0] += 1
                xb[s] = xb_pool.tile([128, 4096], BF16, tag="xb",
                                     name=f"xb_{uid[0]}")
                in_dma(xb[s][:, 0:2048], x_d[n * N_STRIPS + s, :, 0:2048])
                in_dma(xb[s][:, 2048:4096], x_d[n * N_STRIPS + s, :, 2048:4096])
                uid[0] += 1
                ost[s] = out_pool.tile([128, 4080], BF16, tag="ostrip",
                                       name=f"os_{uid[0]}")
                if s >= 1:
                    for q in range(8):
                        emit_quad(q, xb[s - 1], xb[s] if q == 7 else None,
                                  ost[s - 1])
                    out_dma(o_d[n * N_STRIPS + s - 1], ost[s - 1][:])
            s = N_STRIPS - 1
            for q in range(8):
                emit_quad(q, xb[s], None, ost[s], nrow=4 if q < 7 else 2)
            idx = n * N_STRIPS + s
            out_dma(o_d[idx, 0:64], ost[s][0:64, :])
            out_dma(o_d[idx, 64:128, 0:3570], ost[s][64:128, 0:3570])

    nc.compile()
    return nc


def _prep_x(x):
    """[16, 32, 512, 512] f32 -> per-core list of [32, 128, 4096] bf16."""
    xb = x.astype(BF)
    cores = []
    for c in range(N_CORES):
        imgs = []
        for n in range(N_PER):
            im = xb[c * N_PER + n]                      # [32, 512, 512]
            im = im.reshape(C, N_STRIPS, 8, 4, W)       # ci, s, k, g, w
            im = im.transpose(1, 3, 0, 2, 4)            # s, g, ci, k, w
            imgs.append(np.ascontiguousarray(im.reshape(N_STRIPS, 128, 4096)))
        cores.append(np.concatenate(imgs, axis=0))
    return cores


def _prep_w(weight):
    """[32, 32, 3, 3] f32 -> [128, 384] bf16.
    Column layout: 32*(ab*6 + kw*2 + colhalf); both K-halves (partitions
    0-63 and 64-127) carry the same content.
      A: [[kh0, 0], [kh1, kh0]]   (K-half x col-half)
      B: [[kh2, kh1], [0, kh2]]
    """
    wb = np.zeros((128, 384), dtype=np.float32)
    wt = {kh: weight[:, :, kh, :] for kh in range(3)}
    for kw in range(3):
        for T in (0, 64):
            for ab, pat in ((0, ((0, None), (1, 0))), (1, ((2, 1), (None, 2)))):
                c0 = 32 * (ab * 6 + kw * 2)
                for gl in range(2):          # K sub-half (g_lo, g_hi)
                    for ch in range(2):      # col half (out row 0/1 of pair)
                        kh = pat[gl][ch]
                        if kh is not None:
                            wb[T + 32 * gl:T + 32 * gl + 32,
                               c0 + 32 * ch:c0 + 32 * ch + 32] = \
                                weight[:, :, kh, kw].T
    return wb.astype(BF)


def _unprep_out(o_arrs):
    """per-core [32, 128, 4080] bf16 -> [16, 32, 510, 510] f32."""
    full = np.empty((N_FULL, C, HO, WO), dtype=np.float32)
    for c, arr in enumerate(o_arrs):
        a = np.asarray(arr).reshape(N_PER, N_STRIPS, 4, 32, 8, WO)
        # dims: n, s, p, co, q, w  ->  n, co, s, q, p, w
        a = a.transpose(0, 3, 1, 4, 2, 5).reshape(N_PER, C, 512, WO)
        full[c * N_PER:(c + 1) * N_PER] = a[:, :, :HO, :].astype(np.float32)
    return full


_NC = None


def prepare_in_maps(x, weight, bias):
    x = np.ascontiguousarray(np.asarray(x, dtype=np.float32))
    weight = np.ascontiguousarray(np.asarray(weight, dtype=np.float32))
    bias = np.ascontiguousarray(np.asarray(bias, dtype=np.float32))
    xs = _prep_x(x)
    wb = _prep_w(weight)
    bt = np.repeat(bias.reshape(1, 32), 4, axis=0).reshape(128, 1)
    bt = np.ascontiguousarray(bt, dtype=np.float32)
    return [{"xs": xs[i], "wb": wb, "bt": bt} for i in range(N_CORES)]


def kernel(x, weight, bias):
    global _NC
    if _NC is None:
        _NC = _build()
    in_maps = prepare_in_maps(x, weight, bias)
    res = run_bass_kernel_spmd(_NC, in_maps, core_ids=list(range(N_CORES)))
    return _unprep_out([r["out"] for r in res.results])
